# revision 35
# baseline (speedup 1.0000x reference)
"""HQQ 4-bit quantized linear on 8 trn2 NeuronCores (hybrid fp8/fp16).

Computation: out[b,s,o] = sum_i x[b,s,i] * W_est[o,i] + bias[o], where
W_est = ((unpack4bit(W_q) - zero) * scale).reshape(4096, 4096).

Sharding (2 token-halves x 4 output-quarters): core c = 4*h + q computes
out[2048h : 2048h+2048, 1024q : 1024q+1024].  This halves the replicated-x
DMA per core vs pure column-parallel (the PE stream is identical either
way; the baseline's mid-kernel stalls were x-DMA starvation).

Precision: the contraction dim i is split NF8 columns fp8-e4m3 (DoubleRow,
2 MACs/cycle) + the rest fp16.  Everything is scaled by 2^14 (x by 16, W
by 1024 -- lossless powers of 2 for the fp16 side) so fp8 and fp16 matmuls
accumulate into the SAME fp32 PSUM bank; one fused DVE op rescales and
adds bias on drain.  fp8 W values sit in e4m3's normal range (|W|*1024 up
to ~157 < 240); measured end-to-end rel err ~1.5e-2 < 2e-2 gate.

Dequant happens directly in transposed [i, oc] layout (no PE transposes,
no PSUM round-trip): host ships the 4-bit codes Q as e4m3 (integers 0..15
are exact in e4m3) already transposed, plus zero/scale in [i, oc%64]
layout; the device does (Q - z) * s with stride-0 broadcast APs along the
64-periodic oc axis, f16 arithmetic (2x DVE rate), split across the
vector and gpsimd engines.

Device program per core:
  1. Dequant 32 i-planes: NF8/128 planes -> W8T e4m3 [128, *, 1024],
     rest -> W16T f16.
  2. Main: 8 chunks of 256 tokens; per 128-token tile: 2x(NF8/256) fp8
     DoubleRow MMs (stationary x8 plane-pair, moving W8T [128,2,512]) +
     2x24 fp16 MMs (stationary x16 [128,128], moving W16T [128,512]),
     all accumulating into psum[t 128, oc 512]; drain = fused
     (psum * 2^-14) + bias on DVE, stores [128, 512] f32.
"""

import sys

import numpy as np

try:
    import concourse.bass as bass
except ImportError:  # fresh grading dir: fall back to the repo checkout
    for _p in ("/opt/trn_rl_repo", "/root/.axon_site/_ro/trn_rl_repo"):
        if _p not in sys.path:
            sys.path.insert(0, _p)
    import concourse.bass as bass

import ml_dtypes

import concourse.tile as tile
from concourse import bacc, mybir
from concourse.bass_utils import run_bass_kernel_spmd

# Problem constants (hardcoded per harness contract).
B, S_TOK, IN_F, OUT_F, GROUP = 8, 512, 4096, 4096, 64
T = B * S_TOK                # 4096 tokens
NCORES = 8
TSPLIT, OSPLIT = 2, 4        # core c = 4*h + q
TLOC = T // TSPLIT           # 2048 tokens per core
OC = OUT_F // OSPLIT         # 1024 output features per core
NG = IN_F * OUT_F // GROUP   # 262144 quant groups

NF8 = 1024                   # contraction columns computed in fp8 (multiple of 256)
NP8 = NF8 // 128             # fp8 i-planes (even)
NP16 = (IN_F - NF8) // 128   # fp16 i-planes
NPL = IN_F // 128            # 32 total i-planes

XSCALE = 16.0                # x pre-scale (power of 2, lossless in fp16)
WSCALE = 1024.0              # W pre-scale
DRAIN = 1.0 / (XSCALE * WSCALE)

TCHUNK = 256                 # tokens per psum round -> 4 banks of [128, 512]
NCH = TLOC // TCHUNK         # 8 chunks

F16 = mybir.dt.float16
F32 = mybir.dt.float32
F8 = mybir.dt.float8e4
E4M3 = ml_dtypes.float8_e4m3


def _trace_body(nc):
    Alu = mybir.AluOpType
    DR = mybir.MatmulPerfMode.DoubleRow
    # x16 ships pre-blocked so every DMA is fully contiguous per partition
    # (24KB lines); strided token-slicing would yield 0.5-1KB descriptor
    # lines that crawl through the DMA queues.
    x16a_d = nc.dram_tensor("x16a", [128, NP16, 512], F16, kind="ExternalInput")
    x16b = nc.dram_tensor("x16b", [TLOC // TCHUNK - 2, 128, NP16, TCHUNK],
                          F16, kind="ExternalInput")
    x8 = nc.dram_tensor("x8", [128, NP8, TLOC], F8, kind="ExternalInput")
    wqt = nc.dram_tensor("wqt", [128, NPL, OC], F8, kind="ExternalInput")
    zzt = nc.dram_tensor("zzt", [128, NPL, 64], F16, kind="ExternalInput")
    sst = nc.dram_tensor("sst", [128, NPL, 64], F16, kind="ExternalInput")
    bias_b = nc.dram_tensor("bias_b", [128, OC], F32, kind="ExternalInput")
    out = nc.dram_tensor("out", [TLOC, OC], F32, kind="ExternalOutput")

    TA = 512                  # phase-A token span (tokens 0:TA, 8 psum banks)
    NOB = OC // 512

    with tile.TileContext(nc) as tc:
        with (
            tc.tile_pool(name="res", bufs=1) as res,
            tc.tile_pool(name="wqp", bufs=6) as wqp,
            tc.tile_pool(name="tmpp", bufs=4) as tmpp,
            tc.tile_pool(name="xcp", bufs=2) as xcp,
            tc.tile_pool(name="outp", bufs=4) as outp,
            tc.tile_pool(name="psp", bufs=8, space=bass.MemorySpace.PSUM) as psp,
        ):
            # --- resident tensors ---
            # Queue roles: sync = pure input pump (zz, wq, x16a, all x16
            # chunks -- nothing on it ever waits except pool pacing);
            # scalar = ss + casts + ALL output stores; gpsimd (slow SWDGE)
            # = x8/bias only, needed late.
            zz_sb = res.tile([128, NPL, 64], F16)
            ss_sb = res.tile([128, NPL, 64], F16)
            nc.scalar.dma_start(zz_sb[:], zzt[:])
            nc.scalar.dma_start(ss_sb[:], sst[:])
            x8_sb = res.tile([128, NP8, TLOC], F8)
            bias_sb = res.tile([128, OC], F32)
            w8t = res.tile([128, NP8, OC], F8)
            w16t = res.tile([128, NP16, OC], F16)
            x16a = res.tile([128, NP16, TA], F16)
            dum = res.tile([128, 512], F16)
            nc.vector.memset(dum[:], 0.0)
            # wq stays fp8 in HBM (half the critical-path DMA) and lands in
            # one resident tile via big sub-DMAs -- no pool recycling, so no
            # DMA trigger ever waits.  Per-plane fp8->f16 casts on the
            # scalar engine (1.15us) feed the 2x-mode vector TTs (0.69us).
            # Interleave wq sub-DMAs with x16a quarters in consumption order.
            wq_all = res.tile([128, NPL, OC], F8)
            nc.sync.dma_start(wq_all[:, 8:12, :], wqt[:, 8:12, :])
            nc.sync.dma_start(x16a[:, 0:6, :], x16a_d[:, 0:6, :])
            nc.sync.dma_start(wq_all[:, 12:16, :], wqt[:, 12:16, :])
            nc.sync.dma_start(x16a[:, 6:12, :], x16a_d[:, 6:12, :])
            nc.sync.dma_start(wq_all[:, 16:20, :], wqt[:, 16:20, :])
            nc.sync.dma_start(x16a[:, 12:NP16, :], x16a_d[:, 12:NP16, :])
            for j0 in (20, 24, 28, 0, 4):
                nc.sync.dma_start(wq_all[:, j0:j0 + 4, :], wqt[:, j0:j0 + 4, :])

            def dequant(j):
                """(Q - z) * s for i-plane j: scalar cast + vector TTs."""
                wq16 = wqp.tile([128, OC], F16, tag="wq16", bufs=4,
                                name=f"wq16_{j}")
                nc.scalar.copy(wq16[:], wq_all[:, j, :])
                zb = zz_sb[:, j, :].unsqueeze(1).broadcast_to([128, OC // 64, 64])
                sb_ = ss_sb[:, j, :].unsqueeze(1).broadcast_to([128, OC // 64, 64])
                wq3 = wq16[:, :].rearrange("p (r m) -> p r m", m=64)
                tmp = tmpp.tile([128, OC], F16, tag="tmp", name=f"tmp{j}")
                tmp3 = tmp[:, :].rearrange("p (r m) -> p r m", m=64)
                nc.vector.tensor_tensor(tmp3, wq3, zb, op=Alu.subtract)
                if j < NP8:
                    # direct fp8-out TT runs at 1x (1.2us) but beats any
                    # cast chain (gpsimd casts cost ~3.9us each).
                    o3 = w8t[:, j, :].rearrange("p (r m) -> p r m", m=64)
                else:
                    o3 = w16t[:, j - NP8, :].rearrange("p (r m) -> p r m", m=64)
                nc.vector.tensor_tensor(o3, tmp3, sb_, op=Alu.mult)

            def drain(ps, t_lo, ob):
                o_sb = outp.tile([128, 512], F32, tag="o")
                nc.vector.scalar_tensor_tensor(
                    o_sb[:], ps[:], DRAIN,
                    bias_sb[:, ob * 512:(ob + 1) * 512],
                    op0=Alu.mult, op1=Alu.add,
                )
                nc.scalar.dma_start(
                    out[t_lo:t_lo + 128, ob * 512:(ob + 1) * 512], o_sb[:])

            # --- phase A: tokens 0:TA, plane-major (PE follows the dequant
            # stream at 8 MMs per plane instead of starving at 2) ---
            psA = [[psp.tile([128, 512], F32, tag="ps", name=f"psA{tt}_{ob}")
                    for ob in range(NOB)] for tt in range(TA // 128)]
            # fp16 planes stream first (consumption-rate matched); fp8
            # planes dequant mid-stream so their MMs are ready well before
            # they close phase A.
            dequant(NP8 + 0)
            dequant(NP8 + 1)
            for jj in range(2, NP16):
                dequant(NP8 + jj)
                if jj == 6:
                    nc.gpsimd.dma_start(x8_sb[:], x8[:])
                    nc.gpsimd.dma_start(bias_sb[:], bias_b[:])
            for j in range(NP8):
                dequant(j)   # fp8 planes last = exact PE consumption order
            # PE warm-up: HAM needs ~3.4us of activity to unthrottle; run
            # dummy matmuls on a zero tile while the first W planes dequant.
            for _ in range(12):
                nc.tensor.matmul(
                    psA[0][0][:], dum[:, 0:128], dum[:, :],
                    start=True, stop=True, skip_group_check=True,
                )
            for it in range(NP16):
                for tt in range(TA // 128):
                    for ob in range(NOB):
                        nc.tensor.matmul(
                            psA[tt][ob][:],
                            x16a[:, it, tt * 128:tt * 128 + 128],
                            w16t[:, it, ob * 512:(ob + 1) * 512],
                            start=(it == 0), stop=False,
                        )
            for pp in range(0, NP8, 2):
                for tt in range(TA // 128):
                    for ob in range(NOB):
                        nc.tensor.matmul(
                            psA[tt][ob][:],
                            x8_sb[:, pp:pp + 2, tt * 128:tt * 128 + 128],
                            w8t[:, pp:pp + 2, ob * 512:(ob + 1) * 512],
                            start=False, stop=(pp == NP8 - 2),
                            perf_mode=DR,
                        )
            # pre-issue the first phase-B chunk DMAs so they aren't stuck
            # behind the phase-A drain-store triggers in sync's program
            xcs = {}
            for ch in range(TA // TCHUNK, min(TA // TCHUNK + 2, NCH)):
                xcs[ch] = xcp.tile([128, NP16, TCHUNK], F16, tag="xc",
                                   name=f"xc{ch}")
                nc.sync.dma_start(xcs[ch][:], x16b[ch - 2])
            for tt in range(TA // 128):
                for ob in range(NOB):
                    drain(psA[tt][ob], tt * 128, ob)

            # --- phase B: remaining tokens, token-major ---
            for ch in range(TA // TCHUNK, NCH):
                if ch in xcs:
                    xc = xcs[ch]
                else:
                    xc = xcp.tile([128, NP16, TCHUNK], F16, tag="xc",
                                  name=f"xc{ch}")
                    nc.sync.dma_start(xc[:], x16b[ch - 2])
                for tt in range(TCHUNK // 128):
                    t0 = ch * TCHUNK + tt * 128
                    psums = [
                        psp.tile([128, 512], F32, tag="ps", name=f"ps{ch}_{tt}_{ob}")
                        for ob in range(NOB)
                    ]
                    for pp in range(0, NP8, 2):
                        for ob in range(NOB):
                            nc.tensor.matmul(
                                psums[ob][:],
                                x8_sb[:, pp:pp + 2, t0:t0 + 128],
                                w8t[:, pp:pp + 2, ob * 512:(ob + 1) * 512],
                                start=(pp == 0), stop=False,
                                perf_mode=DR,
                            )
                    last_tile = (ch == NCH - 1 and tt == TCHUNK // 128 - 1)
                    if last_tile:
                        # ob-major: bank ob0 closes ~5us early so its
                        # drain+store overlaps ob1's matmuls (tail shave)
                        for ob in range(NOB):
                            for it in range(NP16):
                                nc.tensor.matmul(
                                    psums[ob][:],
                                    xc[:, it, tt * 128:tt * 128 + 128],
                                    w16t[:, it, ob * 512:(ob + 1) * 512],
                                    start=False, stop=(it == NP16 - 1),
                                )
                            drain(psums[ob], t0, ob)
                    else:
                        for it in range(NP16):
                            for ob in range(NOB):
                                nc.tensor.matmul(
                                    psums[ob][:],
                                    xc[:, it, tt * 128:tt * 128 + 128],
                                    w16t[:, it, ob * 512:(ob + 1) * 512],
                                    start=False, stop=(it == NP16 - 1),
                                )
                    if not last_tile:
                        for ob in range(NOB):
                            drain(psums[ob], t0, ob)


_CACHED_NC = None


def _get_nc():
    global _CACHED_NC
    if _CACHED_NC is None:
        nc = bacc.Bacc("TRN2", target_bir_lowering=False, debug=False)
        _trace_body(nc)
        nc.compile()
        _CACHED_NC = nc
    return _CACHED_NC


def _plane_pack(a):
    """[TLOC, n*128] -> [128, n, TLOC] with i = j*128 + p."""
    tl, nf = a.shape
    return np.ascontiguousarray(a.reshape(tl, nf // 128, 128).transpose(2, 1, 0))


def make_in_maps(x, W_q, scale, zero, bias):
    """Shard the full inputs into the 8 per-core input maps."""
    xs = np.asarray(x).reshape(T, IN_F).astype(np.float32) * XSCALE
    W_q = np.asarray(W_q)
    # zero/scale in [i, m=oc%64] layout, plane-packed to [128, NPL, 64].
    zz = np.asarray(zero).reshape(GROUP, IN_F).T.astype(np.float16)
    ss = (np.asarray(scale).reshape(GROUP, IN_F).T * WSCALE).astype(np.float16)
    zz_t = np.ascontiguousarray(zz.reshape(NPL, 128, 64).transpose(1, 0, 2))
    ss_t = np.ascontiguousarray(ss.reshape(NPL, 128, 64).transpose(1, 0, 2))
    bias = np.asarray(bias).astype(np.float32)

    x16a_h, x16b_h, x8_h = [], [], []
    for h in range(TSPLIT):
        xh = xs[h * TLOC:(h + 1) * TLOC]
        x16 = _plane_pack(xh[:, NF8:]).astype(np.float16)  # [128, NP16, TLOC]
        x16a_h.append(np.ascontiguousarray(x16[:, :, 0:512]))
        x16b_h.append(np.ascontiguousarray(
            x16.reshape(128, NP16, NCH, TCHUNK)[:, :, 2:, :].transpose(2, 0, 1, 3)))
        x8_h.append(_plane_pack(xh[:, :NF8]).astype(E4M3))

    wqt_q, bias_q = [], []
    for q in range(OSPLIT):
        g0 = q * (OC // 64)          # first unpacked row for this quarter
        if g0 < GROUP // 2:
            rows = ((W_q[g0:g0 + OC // 64] >> 4) & 15)
        else:
            rows = (W_q[g0 - GROUP // 2:g0 - GROUP // 2 + OC // 64] & 15)
        # rows: [16, NG] -> Q[oc_l, i] with oc_l = g_l*64 + m, col n = m*4096 + i
        Qm = rows.reshape(OC // 64, 64, IN_F).reshape(OC, IN_F)
        QT = Qm.T.astype(np.float32)                    # [i, oc_l]
        wqt_q.append(np.ascontiguousarray(
            QT.reshape(NPL, 128, OC).transpose(1, 0, 2)).astype(E4M3))
        bias_q.append(np.ascontiguousarray(
            np.broadcast_to(bias[OC * q:OC * (q + 1)], (128, OC))))

    in_maps = []
    for c in range(NCORES):
        h, q = c // OSPLIT, c % OSPLIT
        in_maps.append({
            "x16a": x16a_h[h],
            "x16b": x16b_h[h],
            "x8": x8_h[h],
            "wqt": wqt_q[q],
            "zzt": zz_t,
            "sst": ss_t,
            "bias_b": bias_q[q],
        })
    return in_maps


def assemble(results):
    """results: list of per-core {"out": [TLOC, OC] f32} -> [B, S, OUT_F] f32."""
    full = np.empty((T, OUT_F), np.float32)
    for c in range(NCORES):
        h, q = c // OSPLIT, c % OSPLIT
        full[h * TLOC:(h + 1) * TLOC, q * OC:(q + 1) * OC] = results[c]["out"]
    return full.reshape(B, S_TOK, OUT_F)


def kernel(x, W_q, scale, zero, bias):
    nc = _get_nc()
    in_maps = make_in_maps(x, W_q, scale, zero, bias)
    res = run_bass_kernel_spmd(nc, in_maps, core_ids=list(range(NCORES)))
    return assemble(res.results)


if __name__ == "__main__":
    # Quick CoreSim check of cores 0 and 7 against a numpy reference.
    from concourse.bass_interp import CoreSim

    rng = np.random.default_rng(0)
    x = rng.standard_normal((B, S_TOK, IN_F), dtype=np.float32)
    W_q = rng.integers(0, 256, (GROUP // 2, NG)).astype(np.int32)
    scale = rng.uniform(1e-3, 1e-2, (1, NG)).astype(np.float32)
    zero = rng.uniform(0.0, 15.0, (1, NG)).astype(np.float32)
    bias = (rng.standard_normal(OUT_F) * 0.01).astype(np.float32)

    hi = (W_q >> 4) & 0xF
    lo = W_q & 0xF
    W_p = np.concatenate([hi, lo], axis=0).astype(np.float32)
    W_est = ((W_p - zero) * scale).reshape(OUT_F, IN_F)
    ref = x.reshape(T, IN_F) @ W_est.T + bias

    nc = _get_nc()
    in_maps = make_in_maps(x, W_q, scale, zero, bias)
    for core in (0, 7):
        sim = CoreSim(nc, trace=False)
        for k, v in in_maps[core].items():
            sim.tensor(k)[:] = v
        sim.simulate(check_with_hw=False)
        got = np.asarray(sim.tensor("out"))
        h, q = core // OSPLIT, core % OSPLIT
        exp = ref[h * TLOC:(h + 1) * TLOC, q * OC:(q + 1) * OC]
        err = np.abs(got - exp)
        rel = err.max() / np.abs(ref).max()
        print(f"core {core}: max abs err {err.max():.3e}  "
              f"rel (vs global absmax) {rel:.3e}  mean abs {err.mean():.3e}")


# revision 37
# speedup vs baseline: 1.0149x; 1.0149x over previous
"""HQQ 4-bit quantized linear on 8 trn2 NeuronCores (hybrid fp8/fp16).

Computation: out[b,s,o] = sum_i x[b,s,i] * W_est[o,i] + bias[o], where
W_est = ((unpack4bit(W_q) - zero) * scale).reshape(4096, 4096).

Sharding (2 token-halves x 4 output-quarters): core c = 4*h + q computes
out[2048h : 2048h+2048, 1024q : 1024q+1024].  This halves the replicated-x
DMA per core vs pure column-parallel (the PE stream is identical either
way; the baseline's mid-kernel stalls were x-DMA starvation).

Precision: the contraction dim i is split NF8 columns fp8-e4m3 (DoubleRow,
2 MACs/cycle) + the rest fp16.  Everything is scaled by 2^14 (x by 16, W
by 1024 -- lossless powers of 2 for the fp16 side) so fp8 and fp16 matmuls
accumulate into the SAME fp32 PSUM bank; one fused DVE op rescales and
adds bias on drain.  fp8 W values sit in e4m3's normal range (|W|*1024 up
to ~157 < 240); measured end-to-end rel err ~1.5e-2 < 2e-2 gate.

Dequant happens directly in transposed [i, oc] layout (no PE transposes,
no PSUM round-trip): host ships the 4-bit codes Q as e4m3 (integers 0..15
are exact in e4m3) already transposed, plus zero/scale in [i, oc%64]
layout; the device does (Q - z) * s with stride-0 broadcast APs along the
64-periodic oc axis, f16 arithmetic (2x DVE rate), split across the
vector and gpsimd engines.

Device program per core:
  1. Dequant 32 i-planes: NF8/128 planes -> W8T e4m3 [128, *, 1024],
     rest -> W16T f16.
  2. Main: 8 chunks of 256 tokens; per 128-token tile: 2x(NF8/256) fp8
     DoubleRow MMs (stationary x8 plane-pair, moving W8T [128,2,512]) +
     2x24 fp16 MMs (stationary x16 [128,128], moving W16T [128,512]),
     all accumulating into psum[t 128, oc 512]; drain = fused
     (psum * 2^-14) + bias on DVE, stores [128, 512] f32.
"""

import sys

import numpy as np

try:
    import concourse.bass as bass
except ImportError:  # fresh grading dir: fall back to the repo checkout
    for _p in ("/opt/trn_rl_repo", "/root/.axon_site/_ro/trn_rl_repo"):
        if _p not in sys.path:
            sys.path.insert(0, _p)
    import concourse.bass as bass

import ml_dtypes

import concourse.tile as tile
from concourse import bacc, mybir
from concourse.bass_utils import run_bass_kernel_spmd

# Problem constants (hardcoded per harness contract).
B, S_TOK, IN_F, OUT_F, GROUP = 8, 512, 4096, 4096, 64
T = B * S_TOK                # 4096 tokens
NCORES = 8
TSPLIT, OSPLIT = 2, 4        # core c = 4*h + q
TLOC = T // TSPLIT           # 2048 tokens per core
OC = OUT_F // OSPLIT         # 1024 output features per core
NG = IN_F * OUT_F // GROUP   # 262144 quant groups

NF8 = 1024                   # contraction columns computed in fp8 (multiple of 256)
NP8 = NF8 // 128             # fp8 i-planes (even)
NP16 = (IN_F - NF8) // 128   # fp16 i-planes
NPL = IN_F // 128            # 32 total i-planes

XSCALE = 16.0                # x pre-scale (power of 2, lossless in fp16)
WSCALE = 1024.0              # W pre-scale
DRAIN = 1.0 / (XSCALE * WSCALE)

TCHUNK = 256                 # tokens per psum round -> 4 banks of [128, 512]
NCH = TLOC // TCHUNK         # 8 chunks

F16 = mybir.dt.float16
F32 = mybir.dt.float32
F8 = mybir.dt.float8e4
E4M3 = ml_dtypes.float8_e4m3


def _trace_body(nc):
    Alu = mybir.AluOpType
    DR = mybir.MatmulPerfMode.DoubleRow
    # x16 ships pre-blocked so every DMA is fully contiguous per partition
    # (24KB lines); strided token-slicing would yield 0.5-1KB descriptor
    # lines that crawl through the DMA queues.
    x16a_d = nc.dram_tensor("x16a", [128, NP16, 512], F16, kind="ExternalInput")
    x16b = nc.dram_tensor("x16b", [TLOC // TCHUNK - 2, 128, NP16, TCHUNK],
                          F16, kind="ExternalInput")
    x8 = nc.dram_tensor("x8", [128, NP8, TLOC], F8, kind="ExternalInput")
    wqt = nc.dram_tensor("wqt", [128, NPL, OC], F8, kind="ExternalInput")
    zzt = nc.dram_tensor("zzt", [128, NPL, 64], F16, kind="ExternalInput")
    sst = nc.dram_tensor("sst", [128, NPL, 64], F16, kind="ExternalInput")
    bias_b = nc.dram_tensor("bias_b", [128, OC], F32, kind="ExternalInput")
    out = nc.dram_tensor("out", [TLOC, OC], F32, kind="ExternalOutput")

    TA = 512                  # phase-A token span (tokens 0:TA, 8 psum banks)
    NOB = OC // 512

    with tile.TileContext(nc) as tc:
        with (
            tc.tile_pool(name="res", bufs=1) as res,
            tc.tile_pool(name="wqp", bufs=6) as wqp,
            tc.tile_pool(name="tmpp", bufs=4) as tmpp,
            tc.tile_pool(name="xcp", bufs=2) as xcp,
            tc.tile_pool(name="outp", bufs=4) as outp,
            tc.tile_pool(name="psp", bufs=8, space=bass.MemorySpace.PSUM) as psp,
        ):
            # --- resident tensors ---
            # Queue roles: sync = pure input pump (zz, wq, x16a, all x16
            # chunks -- nothing on it ever waits except pool pacing);
            # scalar = ss + casts + ALL output stores; gpsimd (slow SWDGE)
            # = x8/bias only, needed late.
            zz_sb = res.tile([128, NPL, 64], F16)
            ss_sb = res.tile([128, NPL, 64], F16)
            nc.scalar.dma_start(zz_sb[:], zzt[:])
            nc.gpsimd.dma_start(ss_sb[:], sst[:])
            x8_sb = res.tile([128, NP8, TLOC], F8)
            bias_sb = res.tile([128, OC], F32)
            w8t = res.tile([128, NP8, OC], F8)
            w16t = res.tile([128, NP16, OC], F16)
            x16a = res.tile([128, NP16, TA], F16)
            dum = res.tile([128, 512], F16)
            nc.vector.memset(dum[:], 0.0)
            # wq stays fp8 in HBM (half the critical-path DMA) and lands in
            # one resident tile via big sub-DMAs -- no pool recycling, so no
            # DMA trigger ever waits.  Per-plane fp8->f16 casts on the
            # scalar engine (1.15us) feed the 2x-mode vector TTs (0.69us).
            # Interleave wq sub-DMAs with x16a quarters in consumption order.
            wq_all = res.tile([128, NPL, OC], F8)
            nc.sync.dma_start(wq_all[:, 8:10, :], wqt[:, 8:10, :])
            nc.gpsimd.dma_start(wq_all[:, 10:12, :], wqt[:, 10:12, :])
            nc.sync.dma_start(x16a[:, 0:2, :], x16a_d[:, 0:2, :])
            nc.sync.dma_start(wq_all[:, 12:16, :], wqt[:, 12:16, :])
            nc.sync.dma_start(x16a[:, 2:6, :], x16a_d[:, 2:6, :])
            nc.sync.dma_start(wq_all[:, 16:20, :], wqt[:, 16:20, :])
            nc.sync.dma_start(x16a[:, 6:12, :], x16a_d[:, 6:12, :])
            nc.sync.dma_start(wq_all[:, 20:24, :], wqt[:, 20:24, :])
            nc.sync.dma_start(x16a[:, 12:NP16, :], x16a_d[:, 12:NP16, :])
            for j0 in (24, 28, 0, 4):
                nc.sync.dma_start(wq_all[:, j0:j0 + 4, :], wqt[:, j0:j0 + 4, :])

            def dequant(j):
                """(Q - z) * s for i-plane j: scalar cast + vector TTs."""
                wq16 = wqp.tile([128, OC], F16, tag="wq16", bufs=4,
                                name=f"wq16_{j}")
                nc.scalar.copy(wq16[:], wq_all[:, j, :])
                zb = zz_sb[:, j, :].unsqueeze(1).broadcast_to([128, OC // 64, 64])
                sb_ = ss_sb[:, j, :].unsqueeze(1).broadcast_to([128, OC // 64, 64])
                wq3 = wq16[:, :].rearrange("p (r m) -> p r m", m=64)
                tmp = tmpp.tile([128, OC], F16, tag="tmp", name=f"tmp{j}")
                tmp3 = tmp[:, :].rearrange("p (r m) -> p r m", m=64)
                nc.vector.tensor_tensor(tmp3, wq3, zb, op=Alu.subtract)
                if j < NP8:
                    # direct fp8-out TT runs at 1x (1.2us) but beats any
                    # cast chain (gpsimd casts cost ~3.9us each).
                    o3 = w8t[:, j, :].rearrange("p (r m) -> p r m", m=64)
                else:
                    o3 = w16t[:, j - NP8, :].rearrange("p (r m) -> p r m", m=64)
                nc.vector.tensor_tensor(o3, tmp3, sb_, op=Alu.mult)

            def drain(ps, t_lo, ob):
                o_sb = outp.tile([128, 512], F32, tag="o")
                nc.vector.scalar_tensor_tensor(
                    o_sb[:], ps[:], DRAIN,
                    bias_sb[:, ob * 512:(ob + 1) * 512],
                    op0=Alu.mult, op1=Alu.add,
                )
                nc.scalar.dma_start(
                    out[t_lo:t_lo + 128, ob * 512:(ob + 1) * 512], o_sb[:])

            # --- phase A: tokens 0:TA, plane-major (PE follows the dequant
            # stream at 8 MMs per plane instead of starving at 2) ---
            psA = [[psp.tile([128, 512], F32, tag="ps", name=f"psA{tt}_{ob}")
                    for ob in range(NOB)] for tt in range(TA // 128)]
            # fp16 planes stream first (consumption-rate matched); fp8
            # planes dequant mid-stream so their MMs are ready well before
            # they close phase A.
            dequant(NP8 + 0)
            dequant(NP8 + 1)
            for jj in range(2, NP16):
                dequant(NP8 + jj)
                if jj == 6:
                    nc.gpsimd.dma_start(x8_sb[:], x8[:])
                    nc.gpsimd.dma_start(bias_sb[:], bias_b[:])
            for j in range(NP8):
                dequant(j)   # fp8 planes last = exact PE consumption order
            # PE warm-up: HAM needs ~3.4us of activity to unthrottle; run
            # dummy matmuls on a zero tile while the first W planes dequant.
            for _ in range(12):
                nc.tensor.matmul(
                    psA[0][0][:], dum[:, 0:128], dum[:, :],
                    start=True, stop=True, skip_group_check=True,
                )
            for it in range(NP16):
                for tt in range(TA // 128):
                    for ob in range(NOB):
                        nc.tensor.matmul(
                            psA[tt][ob][:],
                            x16a[:, it, tt * 128:tt * 128 + 128],
                            w16t[:, it, ob * 512:(ob + 1) * 512],
                            start=(it == 0), stop=False,
                        )
            for pp in range(0, NP8, 2):
                for tt in range(TA // 128):
                    for ob in range(NOB):
                        nc.tensor.matmul(
                            psA[tt][ob][:],
                            x8_sb[:, pp:pp + 2, tt * 128:tt * 128 + 128],
                            w8t[:, pp:pp + 2, ob * 512:(ob + 1) * 512],
                            start=False, stop=(pp == NP8 - 2),
                            perf_mode=DR,
                        )
            # pre-issue the first phase-B chunk DMAs so they aren't stuck
            # behind the phase-A drain-store triggers in sync's program
            xcs = {}
            for ch in range(TA // TCHUNK, min(TA // TCHUNK + 2, NCH)):
                xcs[ch] = xcp.tile([128, NP16, TCHUNK], F16, tag="xc",
                                   name=f"xc{ch}")
                nc.sync.dma_start(xcs[ch][:], x16b[ch - 2])
            for tt in range(TA // 128):
                for ob in range(NOB):
                    drain(psA[tt][ob], tt * 128, ob)

            # --- phase B: remaining tokens, token-major ---
            for ch in range(TA // TCHUNK, NCH):
                if ch in xcs:
                    xc = xcs[ch]
                else:
                    xc = xcp.tile([128, NP16, TCHUNK], F16, tag="xc",
                                  name=f"xc{ch}")
                    nc.sync.dma_start(xc[:], x16b[ch - 2])
                for tt in range(TCHUNK // 128):
                    t0 = ch * TCHUNK + tt * 128
                    psums = [
                        psp.tile([128, 512], F32, tag="ps", name=f"ps{ch}_{tt}_{ob}")
                        for ob in range(NOB)
                    ]
                    for pp in range(0, NP8, 2):
                        for ob in range(NOB):
                            nc.tensor.matmul(
                                psums[ob][:],
                                x8_sb[:, pp:pp + 2, t0:t0 + 128],
                                w8t[:, pp:pp + 2, ob * 512:(ob + 1) * 512],
                                start=(pp == 0), stop=False,
                                perf_mode=DR,
                            )
                    last_tile = (ch == NCH - 1 and tt == TCHUNK // 128 - 1)
                    if last_tile:
                        # ob-major: bank ob0 closes ~5us early so its
                        # drain+store overlaps ob1's matmuls (tail shave)
                        for ob in range(NOB):
                            for it in range(NP16):
                                nc.tensor.matmul(
                                    psums[ob][:],
                                    xc[:, it, tt * 128:tt * 128 + 128],
                                    w16t[:, it, ob * 512:(ob + 1) * 512],
                                    start=False, stop=(it == NP16 - 1),
                                )
                            drain(psums[ob], t0, ob)
                    else:
                        for it in range(NP16):
                            for ob in range(NOB):
                                nc.tensor.matmul(
                                    psums[ob][:],
                                    xc[:, it, tt * 128:tt * 128 + 128],
                                    w16t[:, it, ob * 512:(ob + 1) * 512],
                                    start=False, stop=(it == NP16 - 1),
                                )
                    if not last_tile:
                        for ob in range(NOB):
                            drain(psums[ob], t0, ob)


_CACHED_NC = None


def _get_nc():
    global _CACHED_NC
    if _CACHED_NC is None:
        nc = bacc.Bacc("TRN2", target_bir_lowering=False, debug=False)
        _trace_body(nc)
        nc.compile()
        _CACHED_NC = nc
    return _CACHED_NC


def _plane_pack(a):
    """[TLOC, n*128] -> [128, n, TLOC] with i = j*128 + p."""
    tl, nf = a.shape
    return np.ascontiguousarray(a.reshape(tl, nf // 128, 128).transpose(2, 1, 0))


def make_in_maps(x, W_q, scale, zero, bias):
    """Shard the full inputs into the 8 per-core input maps."""
    xs = np.asarray(x).reshape(T, IN_F).astype(np.float32) * XSCALE
    W_q = np.asarray(W_q)
    # zero/scale in [i, m=oc%64] layout, plane-packed to [128, NPL, 64].
    zz = np.asarray(zero).reshape(GROUP, IN_F).T.astype(np.float16)
    ss = (np.asarray(scale).reshape(GROUP, IN_F).T * WSCALE).astype(np.float16)
    zz_t = np.ascontiguousarray(zz.reshape(NPL, 128, 64).transpose(1, 0, 2))
    ss_t = np.ascontiguousarray(ss.reshape(NPL, 128, 64).transpose(1, 0, 2))
    bias = np.asarray(bias).astype(np.float32)

    x16a_h, x16b_h, x8_h = [], [], []
    for h in range(TSPLIT):
        xh = xs[h * TLOC:(h + 1) * TLOC]
        x16 = _plane_pack(xh[:, NF8:]).astype(np.float16)  # [128, NP16, TLOC]
        x16a_h.append(np.ascontiguousarray(x16[:, :, 0:512]))
        x16b_h.append(np.ascontiguousarray(
            x16.reshape(128, NP16, NCH, TCHUNK)[:, :, 2:, :].transpose(2, 0, 1, 3)))
        x8_h.append(_plane_pack(xh[:, :NF8]).astype(E4M3))

    wqt_q, bias_q = [], []
    for q in range(OSPLIT):
        g0 = q * (OC // 64)          # first unpacked row for this quarter
        if g0 < GROUP // 2:
            rows = ((W_q[g0:g0 + OC // 64] >> 4) & 15)
        else:
            rows = (W_q[g0 - GROUP // 2:g0 - GROUP // 2 + OC // 64] & 15)
        # rows: [16, NG] -> Q[oc_l, i] with oc_l = g_l*64 + m, col n = m*4096 + i
        Qm = rows.reshape(OC // 64, 64, IN_F).reshape(OC, IN_F)
        QT = Qm.T.astype(np.float32)                    # [i, oc_l]
        wqt_q.append(np.ascontiguousarray(
            QT.reshape(NPL, 128, OC).transpose(1, 0, 2)).astype(E4M3))
        bias_q.append(np.ascontiguousarray(
            np.broadcast_to(bias[OC * q:OC * (q + 1)], (128, OC))))

    in_maps = []
    for c in range(NCORES):
        h, q = c // OSPLIT, c % OSPLIT
        in_maps.append({
            "x16a": x16a_h[h],
            "x16b": x16b_h[h],
            "x8": x8_h[h],
            "wqt": wqt_q[q],
            "zzt": zz_t,
            "sst": ss_t,
            "bias_b": bias_q[q],
        })
    return in_maps


def assemble(results):
    """results: list of per-core {"out": [TLOC, OC] f32} -> [B, S, OUT_F] f32."""
    full = np.empty((T, OUT_F), np.float32)
    for c in range(NCORES):
        h, q = c // OSPLIT, c % OSPLIT
        full[h * TLOC:(h + 1) * TLOC, q * OC:(q + 1) * OC] = results[c]["out"]
    return full.reshape(B, S_TOK, OUT_F)


def kernel(x, W_q, scale, zero, bias):
    nc = _get_nc()
    in_maps = make_in_maps(x, W_q, scale, zero, bias)
    res = run_bass_kernel_spmd(nc, in_maps, core_ids=list(range(NCORES)))
    return assemble(res.results)


if __name__ == "__main__":
    # Quick CoreSim check of cores 0 and 7 against a numpy reference.
    from concourse.bass_interp import CoreSim

    rng = np.random.default_rng(0)
    x = rng.standard_normal((B, S_TOK, IN_F), dtype=np.float32)
    W_q = rng.integers(0, 256, (GROUP // 2, NG)).astype(np.int32)
    scale = rng.uniform(1e-3, 1e-2, (1, NG)).astype(np.float32)
    zero = rng.uniform(0.0, 15.0, (1, NG)).astype(np.float32)
    bias = (rng.standard_normal(OUT_F) * 0.01).astype(np.float32)

    hi = (W_q >> 4) & 0xF
    lo = W_q & 0xF
    W_p = np.concatenate([hi, lo], axis=0).astype(np.float32)
    W_est = ((W_p - zero) * scale).reshape(OUT_F, IN_F)
    ref = x.reshape(T, IN_F) @ W_est.T + bias

    nc = _get_nc()
    in_maps = make_in_maps(x, W_q, scale, zero, bias)
    for core in (0, 7):
        sim = CoreSim(nc, trace=False)
        for k, v in in_maps[core].items():
            sim.tensor(k)[:] = v
        sim.simulate(check_with_hw=False)
        got = np.asarray(sim.tensor("out"))
        h, q = core // OSPLIT, core % OSPLIT
        exp = ref[h * TLOC:(h + 1) * TLOC, q * OC:(q + 1) * OC]
        err = np.abs(got - exp)
        rel = err.max() / np.abs(ref).max()
        print(f"core {core}: max abs err {err.max():.3e}  "
              f"rel (vs global absmax) {rel:.3e}  mean abs {err.mean():.3e}")


# revision 39
# speedup vs baseline: 1.0210x; 1.0060x over previous
"""HQQ 4-bit quantized linear on 8 trn2 NeuronCores (hybrid fp8/fp16).

Computation: out[b,s,o] = sum_i x[b,s,i] * W_est[o,i] + bias[o], where
W_est = ((unpack4bit(W_q) - zero) * scale).reshape(4096, 4096).

Sharding (2 token-halves x 4 output-quarters): core c = 4*h + q computes
out[2048h : 2048h+2048, 1024q : 1024q+1024].  This halves the replicated-x
DMA per core vs pure column-parallel (the PE stream is identical either
way; the baseline's mid-kernel stalls were x-DMA starvation).

Precision: the contraction dim i is split NF8 columns fp8-e4m3 (DoubleRow,
2 MACs/cycle) + the rest fp16.  Everything is scaled by 2^14 (x by 16, W
by 1024 -- lossless powers of 2 for the fp16 side) so fp8 and fp16 matmuls
accumulate into the SAME fp32 PSUM bank; one fused DVE op rescales and
adds bias on drain.  fp8 W values sit in e4m3's normal range (|W|*1024 up
to ~157 < 240); measured end-to-end rel err ~1.5e-2 < 2e-2 gate.

Dequant happens directly in transposed [i, oc] layout (no PE transposes,
no PSUM round-trip): host ships the 4-bit codes Q as e4m3 (integers 0..15
are exact in e4m3) already transposed, plus zero/scale in [i, oc%64]
layout; the device does (Q - z) * s with stride-0 broadcast APs along the
64-periodic oc axis, f16 arithmetic (2x DVE rate), split across the
vector and gpsimd engines.

Device program per core:
  1. Dequant 32 i-planes: NF8/128 planes -> W8T e4m3 [128, *, 1024],
     rest -> W16T f16.
  2. Main: 8 chunks of 256 tokens; per 128-token tile: 2x(NF8/256) fp8
     DoubleRow MMs (stationary x8 plane-pair, moving W8T [128,2,512]) +
     2x24 fp16 MMs (stationary x16 [128,128], moving W16T [128,512]),
     all accumulating into psum[t 128, oc 512]; drain = fused
     (psum * 2^-14) + bias on DVE, stores [128, 512] f32.
"""

import sys

import numpy as np

try:
    import concourse.bass as bass
except ImportError:  # fresh grading dir: fall back to the repo checkout
    for _p in ("/opt/trn_rl_repo", "/root/.axon_site/_ro/trn_rl_repo"):
        if _p not in sys.path:
            sys.path.insert(0, _p)
    import concourse.bass as bass

import ml_dtypes

import concourse.tile as tile
from concourse import bacc, mybir
from concourse.bass_utils import run_bass_kernel_spmd

# Problem constants (hardcoded per harness contract).
B, S_TOK, IN_F, OUT_F, GROUP = 8, 512, 4096, 4096, 64
T = B * S_TOK                # 4096 tokens
NCORES = 8
TSPLIT, OSPLIT = 2, 4        # core c = 4*h + q
TLOC = T // TSPLIT           # 2048 tokens per core
OC = OUT_F // OSPLIT         # 1024 output features per core
NG = IN_F * OUT_F // GROUP   # 262144 quant groups

NF8 = 1024                   # contraction columns computed in fp8 (multiple of 256)
NP8 = NF8 // 128             # fp8 i-planes (even)
NP16 = (IN_F - NF8) // 128   # fp16 i-planes
NPL = IN_F // 128            # 32 total i-planes

XSCALE = 16.0                # x pre-scale (power of 2, lossless in fp16)
WSCALE = 1024.0              # W pre-scale
DRAIN = 1.0 / (XSCALE * WSCALE)

TCHUNK = 256                 # tokens per psum round -> 4 banks of [128, 512]
NCH = TLOC // TCHUNK         # 8 chunks

F16 = mybir.dt.float16
F32 = mybir.dt.float32
F8 = mybir.dt.float8e4
E4M3 = ml_dtypes.float8_e4m3


def _trace_body(nc):
    Alu = mybir.AluOpType
    DR = mybir.MatmulPerfMode.DoubleRow
    # x16 ships pre-blocked so every DMA is fully contiguous per partition
    # (24KB lines); strided token-slicing would yield 0.5-1KB descriptor
    # lines that crawl through the DMA queues.
    x16a_d = nc.dram_tensor("x16a", [128, NP16, 512], F16, kind="ExternalInput")
    x16b = nc.dram_tensor("x16b", [TLOC // TCHUNK - 2, 128, NP16, TCHUNK],
                          F16, kind="ExternalInput")
    x8 = nc.dram_tensor("x8", [128, NP8, TLOC], F8, kind="ExternalInput")
    wqt = nc.dram_tensor("wqt", [128, NPL, OC], F8, kind="ExternalInput")
    zzt = nc.dram_tensor("zzt", [128, NPL, 64], F16, kind="ExternalInput")
    sst = nc.dram_tensor("sst", [128, NPL, 64], F16, kind="ExternalInput")
    bias_b = nc.dram_tensor("bias_b", [128, OC], F32, kind="ExternalInput")
    out = nc.dram_tensor("out", [TLOC, OC], F32, kind="ExternalOutput")

    TA = 512                  # phase-A token span (tokens 0:TA, 8 psum banks)
    NOB = OC // 512

    with tile.TileContext(nc) as tc:
        with (
            tc.tile_pool(name="res", bufs=1) as res,
            tc.tile_pool(name="wqp", bufs=6) as wqp,
            tc.tile_pool(name="tmpp", bufs=4) as tmpp,
            tc.tile_pool(name="xcp", bufs=2) as xcp,
            tc.tile_pool(name="outp", bufs=4) as outp,
            tc.tile_pool(name="psp", bufs=8, space=bass.MemorySpace.PSUM) as psp,
        ):
            # --- resident tensors ---
            # Queue roles: sync = pure input pump (zz, wq, x16a, all x16
            # chunks -- nothing on it ever waits except pool pacing);
            # scalar = ss + casts + ALL output stores; gpsimd (slow SWDGE)
            # = x8/bias only, needed late.
            zz_sb = res.tile([128, NPL, 64], F16)
            ss_sb = res.tile([128, NPL, 64], F16)
            nc.scalar.dma_start(zz_sb[:, 8:16, :], zzt[:, 8:16, :])
            nc.scalar.dma_start(ss_sb[:, 8:16, :], sst[:, 8:16, :])
            x8_sb = res.tile([128, NP8, TLOC], F8)
            bias_sb = res.tile([128, OC], F32)
            w8t = res.tile([128, NP8, OC], F8)
            w16t = res.tile([128, NP16, OC], F16)
            x16a = res.tile([128, NP16, TA], F16)
            dum = res.tile([128, 512], F16)
            nc.vector.memset(dum[:], 0.0)
            # wq stays fp8 in HBM (half the critical-path DMA) and lands in
            # one resident tile via big sub-DMAs -- no pool recycling, so no
            # DMA trigger ever waits.  Per-plane fp8->f16 casts on the
            # scalar engine (1.15us) feed the 2x-mode vector TTs (0.69us).
            # Interleave wq sub-DMAs with x16a quarters in consumption order.
            wq_all = res.tile([128, NPL, OC], F8)
            nc.sync.dma_start(wq_all[:, 8:10, :], wqt[:, 8:10, :])
            nc.gpsimd.dma_start(wq_all[:, 10:12, :], wqt[:, 10:12, :])
            nc.sync.dma_start(x16a[:, 0:2, :], x16a_d[:, 0:2, :])
            nc.sync.dma_start(wq_all[:, 12:16, :], wqt[:, 12:16, :])
            nc.sync.dma_start(x16a[:, 2:6, :], x16a_d[:, 2:6, :])
            nc.sync.dma_start(wq_all[:, 16:20, :], wqt[:, 16:20, :])
            nc.sync.dma_start(x16a[:, 6:12, :], x16a_d[:, 6:12, :])
            nc.sync.dma_start(wq_all[:, 20:24, :], wqt[:, 20:24, :])
            nc.sync.dma_start(x16a[:, 12:NP16, :], x16a_d[:, 12:NP16, :])
            for j0 in (24, 28, 0, 4):
                nc.sync.dma_start(wq_all[:, j0:j0 + 4, :], wqt[:, j0:j0 + 4, :])
            for lo, hi in ((16, 32), (0, 8)):
                nc.scalar.dma_start(zz_sb[:, lo:hi, :], zzt[:, lo:hi, :])
                nc.scalar.dma_start(ss_sb[:, lo:hi, :], sst[:, lo:hi, :])

            def dequant(j):
                """(Q - z) * s for i-plane j: scalar cast + vector TTs."""
                wq16 = wqp.tile([128, OC], F16, tag="wq16", bufs=4,
                                name=f"wq16_{j}")
                nc.scalar.copy(wq16[:], wq_all[:, j, :])
                zb = zz_sb[:, j, :].unsqueeze(1).broadcast_to([128, OC // 64, 64])
                sb_ = ss_sb[:, j, :].unsqueeze(1).broadcast_to([128, OC // 64, 64])
                wq3 = wq16[:, :].rearrange("p (r m) -> p r m", m=64)
                tmp = tmpp.tile([128, OC], F16, tag="tmp", name=f"tmp{j}")
                tmp3 = tmp[:, :].rearrange("p (r m) -> p r m", m=64)
                nc.vector.tensor_tensor(tmp3, wq3, zb, op=Alu.subtract)
                if j < NP8:
                    # direct fp8-out TT runs at 1x (1.2us) but beats any
                    # cast chain (gpsimd casts cost ~3.9us each).
                    o3 = w8t[:, j, :].rearrange("p (r m) -> p r m", m=64)
                else:
                    o3 = w16t[:, j - NP8, :].rearrange("p (r m) -> p r m", m=64)
                nc.vector.tensor_tensor(o3, tmp3, sb_, op=Alu.mult)

            def drain(ps, t_lo, ob):
                o_sb = outp.tile([128, 512], F32, tag="o")
                nc.vector.scalar_tensor_tensor(
                    o_sb[:], ps[:], DRAIN,
                    bias_sb[:, ob * 512:(ob + 1) * 512],
                    op0=Alu.mult, op1=Alu.add,
                )
                nc.scalar.dma_start(
                    out[t_lo:t_lo + 128, ob * 512:(ob + 1) * 512], o_sb[:])

            # --- phase A: tokens 0:TA, plane-major (PE follows the dequant
            # stream at 8 MMs per plane instead of starving at 2) ---
            psA = [[psp.tile([128, 512], F32, tag="ps", name=f"psA{tt}_{ob}")
                    for ob in range(NOB)] for tt in range(TA // 128)]
            # fp16 planes stream first (consumption-rate matched); fp8
            # planes dequant mid-stream so their MMs are ready well before
            # they close phase A.
            dequant(NP8 + 0)
            dequant(NP8 + 1)
            for jj in range(2, NP16):
                dequant(NP8 + jj)
                if jj == 6:
                    nc.gpsimd.dma_start(x8_sb[:], x8[:])
                    nc.gpsimd.dma_start(bias_sb[:], bias_b[:])
            for j in range(NP8):
                dequant(j)   # fp8 planes last = exact PE consumption order
            # PE warm-up: HAM needs ~3.4us of activity to unthrottle; run
            # dummy matmuls on a zero tile while the first W planes dequant.
            for _ in range(12):
                nc.tensor.matmul(
                    psA[0][0][:], dum[:, 0:128], dum[:, :],
                    start=True, stop=True, skip_group_check=True,
                )
            for it in range(NP16):
                for tt in range(TA // 128):
                    for ob in range(NOB):
                        nc.tensor.matmul(
                            psA[tt][ob][:],
                            x16a[:, it, tt * 128:tt * 128 + 128],
                            w16t[:, it, ob * 512:(ob + 1) * 512],
                            start=(it == 0), stop=False,
                        )
            for pp in range(0, NP8, 2):
                for tt in range(TA // 128):
                    for ob in range(NOB):
                        nc.tensor.matmul(
                            psA[tt][ob][:],
                            x8_sb[:, pp:pp + 2, tt * 128:tt * 128 + 128],
                            w8t[:, pp:pp + 2, ob * 512:(ob + 1) * 512],
                            start=False, stop=(pp == NP8 - 2),
                            perf_mode=DR,
                        )
            # pre-issue the first phase-B chunk DMAs so they aren't stuck
            # behind the phase-A drain-store triggers in sync's program
            xcs = {}
            for ch in range(TA // TCHUNK, min(TA // TCHUNK + 2, NCH)):
                xcs[ch] = xcp.tile([128, NP16, TCHUNK], F16, tag="xc",
                                   name=f"xc{ch}")
                nc.sync.dma_start(xcs[ch][:], x16b[ch - 2])
            for tt in range(TA // 128):
                for ob in range(NOB):
                    drain(psA[tt][ob], tt * 128, ob)

            # --- phase B: remaining tokens, token-major ---
            for ch in range(TA // TCHUNK, NCH):
                if ch in xcs:
                    xc = xcs[ch]
                else:
                    xc = xcp.tile([128, NP16, TCHUNK], F16, tag="xc",
                                  name=f"xc{ch}")
                    nc.sync.dma_start(xc[:], x16b[ch - 2])
                for tt in range(TCHUNK // 128):
                    t0 = ch * TCHUNK + tt * 128
                    psums = [
                        psp.tile([128, 512], F32, tag="ps", name=f"ps{ch}_{tt}_{ob}")
                        for ob in range(NOB)
                    ]
                    for pp in range(0, NP8, 2):
                        for ob in range(NOB):
                            nc.tensor.matmul(
                                psums[ob][:],
                                x8_sb[:, pp:pp + 2, t0:t0 + 128],
                                w8t[:, pp:pp + 2, ob * 512:(ob + 1) * 512],
                                start=(pp == 0), stop=False,
                                perf_mode=DR,
                            )
                    last_tile = (ch == NCH - 1 and tt == TCHUNK // 128 - 1)
                    if last_tile:
                        # ob-major: bank ob0 closes ~5us early so its
                        # drain+store overlaps ob1's matmuls (tail shave)
                        for ob in range(NOB):
                            for it in range(NP16):
                                nc.tensor.matmul(
                                    psums[ob][:],
                                    xc[:, it, tt * 128:tt * 128 + 128],
                                    w16t[:, it, ob * 512:(ob + 1) * 512],
                                    start=False, stop=(it == NP16 - 1),
                                )
                            drain(psums[ob], t0, ob)
                    else:
                        for it in range(NP16):
                            for ob in range(NOB):
                                nc.tensor.matmul(
                                    psums[ob][:],
                                    xc[:, it, tt * 128:tt * 128 + 128],
                                    w16t[:, it, ob * 512:(ob + 1) * 512],
                                    start=False, stop=(it == NP16 - 1),
                                )
                    if not last_tile:
                        for ob in range(NOB):
                            drain(psums[ob], t0, ob)


_CACHED_NC = None


def _get_nc():
    global _CACHED_NC
    if _CACHED_NC is None:
        nc = bacc.Bacc("TRN2", target_bir_lowering=False, debug=False)
        _trace_body(nc)
        nc.compile()
        _CACHED_NC = nc
    return _CACHED_NC


def _plane_pack(a):
    """[TLOC, n*128] -> [128, n, TLOC] with i = j*128 + p."""
    tl, nf = a.shape
    return np.ascontiguousarray(a.reshape(tl, nf // 128, 128).transpose(2, 1, 0))


def make_in_maps(x, W_q, scale, zero, bias):
    """Shard the full inputs into the 8 per-core input maps."""
    xs = np.asarray(x).reshape(T, IN_F).astype(np.float32) * XSCALE
    W_q = np.asarray(W_q)
    # zero/scale in [i, m=oc%64] layout, plane-packed to [128, NPL, 64].
    zz = np.asarray(zero).reshape(GROUP, IN_F).T.astype(np.float16)
    ss = (np.asarray(scale).reshape(GROUP, IN_F).T * WSCALE).astype(np.float16)
    zz_t = np.ascontiguousarray(zz.reshape(NPL, 128, 64).transpose(1, 0, 2))
    ss_t = np.ascontiguousarray(ss.reshape(NPL, 128, 64).transpose(1, 0, 2))
    bias = np.asarray(bias).astype(np.float32)

    x16a_h, x16b_h, x8_h = [], [], []
    for h in range(TSPLIT):
        xh = xs[h * TLOC:(h + 1) * TLOC]
        x16 = _plane_pack(xh[:, NF8:]).astype(np.float16)  # [128, NP16, TLOC]
        x16a_h.append(np.ascontiguousarray(x16[:, :, 0:512]))
        x16b_h.append(np.ascontiguousarray(
            x16.reshape(128, NP16, NCH, TCHUNK)[:, :, 2:, :].transpose(2, 0, 1, 3)))
        x8_h.append(_plane_pack(xh[:, :NF8]).astype(E4M3))

    wqt_q, bias_q = [], []
    for q in range(OSPLIT):
        g0 = q * (OC // 64)          # first unpacked row for this quarter
        if g0 < GROUP // 2:
            rows = ((W_q[g0:g0 + OC // 64] >> 4) & 15)
        else:
            rows = (W_q[g0 - GROUP // 2:g0 - GROUP // 2 + OC // 64] & 15)
        # rows: [16, NG] -> Q[oc_l, i] with oc_l = g_l*64 + m, col n = m*4096 + i
        Qm = rows.reshape(OC // 64, 64, IN_F).reshape(OC, IN_F)
        QT = Qm.T.astype(np.float32)                    # [i, oc_l]
        wqt_q.append(np.ascontiguousarray(
            QT.reshape(NPL, 128, OC).transpose(1, 0, 2)).astype(E4M3))
        bias_q.append(np.ascontiguousarray(
            np.broadcast_to(bias[OC * q:OC * (q + 1)], (128, OC))))

    in_maps = []
    for c in range(NCORES):
        h, q = c // OSPLIT, c % OSPLIT
        in_maps.append({
            "x16a": x16a_h[h],
            "x16b": x16b_h[h],
            "x8": x8_h[h],
            "wqt": wqt_q[q],
            "zzt": zz_t,
            "sst": ss_t,
            "bias_b": bias_q[q],
        })
    return in_maps


def assemble(results):
    """results: list of per-core {"out": [TLOC, OC] f32} -> [B, S, OUT_F] f32."""
    full = np.empty((T, OUT_F), np.float32)
    for c in range(NCORES):
        h, q = c // OSPLIT, c % OSPLIT
        full[h * TLOC:(h + 1) * TLOC, q * OC:(q + 1) * OC] = results[c]["out"]
    return full.reshape(B, S_TOK, OUT_F)


def kernel(x, W_q, scale, zero, bias):
    nc = _get_nc()
    in_maps = make_in_maps(x, W_q, scale, zero, bias)
    res = run_bass_kernel_spmd(nc, in_maps, core_ids=list(range(NCORES)))
    return assemble(res.results)


if __name__ == "__main__":
    # Quick CoreSim check of cores 0 and 7 against a numpy reference.
    from concourse.bass_interp import CoreSim

    rng = np.random.default_rng(0)
    x = rng.standard_normal((B, S_TOK, IN_F), dtype=np.float32)
    W_q = rng.integers(0, 256, (GROUP // 2, NG)).astype(np.int32)
    scale = rng.uniform(1e-3, 1e-2, (1, NG)).astype(np.float32)
    zero = rng.uniform(0.0, 15.0, (1, NG)).astype(np.float32)
    bias = (rng.standard_normal(OUT_F) * 0.01).astype(np.float32)

    hi = (W_q >> 4) & 0xF
    lo = W_q & 0xF
    W_p = np.concatenate([hi, lo], axis=0).astype(np.float32)
    W_est = ((W_p - zero) * scale).reshape(OUT_F, IN_F)
    ref = x.reshape(T, IN_F) @ W_est.T + bias

    nc = _get_nc()
    in_maps = make_in_maps(x, W_q, scale, zero, bias)
    for core in (0, 7):
        sim = CoreSim(nc, trace=False)
        for k, v in in_maps[core].items():
            sim.tensor(k)[:] = v
        sim.simulate(check_with_hw=False)
        got = np.asarray(sim.tensor("out"))
        h, q = core // OSPLIT, core % OSPLIT
        exp = ref[h * TLOC:(h + 1) * TLOC, q * OC:(q + 1) * OC]
        err = np.abs(got - exp)
        rel = err.max() / np.abs(ref).max()
        print(f"core {core}: max abs err {err.max():.3e}  "
              f"rel (vs global absmax) {rel:.3e}  mean abs {err.mean():.3e}")


# revision 40
# speedup vs baseline: 1.0351x; 1.0138x over previous
"""HQQ 4-bit quantized linear on 8 trn2 NeuronCores (hybrid fp8/fp16).

Computation: out[b,s,o] = sum_i x[b,s,i] * W_est[o,i] + bias[o], where
W_est = ((unpack4bit(W_q) - zero) * scale).reshape(4096, 4096).

Sharding (2 token-halves x 4 output-quarters): core c = 4*h + q computes
out[2048h : 2048h+2048, 1024q : 1024q+1024].  This halves the replicated-x
DMA per core vs pure column-parallel (the PE stream is identical either
way; the baseline's mid-kernel stalls were x-DMA starvation).

Precision: the contraction dim i is split NF8 columns fp8-e4m3 (DoubleRow,
2 MACs/cycle) + the rest fp16.  Everything is scaled by 2^14 (x by 16, W
by 1024 -- lossless powers of 2 for the fp16 side) so fp8 and fp16 matmuls
accumulate into the SAME fp32 PSUM bank; one fused DVE op rescales and
adds bias on drain.  fp8 W values sit in e4m3's normal range (|W|*1024 up
to ~157 < 240); measured end-to-end rel err ~1.5e-2 < 2e-2 gate.

Dequant happens directly in transposed [i, oc] layout (no PE transposes,
no PSUM round-trip): host ships the 4-bit codes Q as e4m3 (integers 0..15
are exact in e4m3) already transposed, plus zero/scale in [i, oc%64]
layout; the device does (Q - z) * s with stride-0 broadcast APs along the
64-periodic oc axis, f16 arithmetic (2x DVE rate), split across the
vector and gpsimd engines.

Device program per core:
  1. Dequant 32 i-planes: NF8/128 planes -> W8T e4m3 [128, *, 1024],
     rest -> W16T f16.
  2. Main: 8 chunks of 256 tokens; per 128-token tile: 2x(NF8/256) fp8
     DoubleRow MMs (stationary x8 plane-pair, moving W8T [128,2,512]) +
     2x24 fp16 MMs (stationary x16 [128,128], moving W16T [128,512]),
     all accumulating into psum[t 128, oc 512]; drain = fused
     (psum * 2^-14) + bias on DVE, stores [128, 512] f32.
"""

import sys

import numpy as np

try:
    import concourse.bass as bass
except ImportError:  # fresh grading dir: fall back to the repo checkout
    for _p in ("/opt/trn_rl_repo", "/root/.axon_site/_ro/trn_rl_repo"):
        if _p not in sys.path:
            sys.path.insert(0, _p)
    import concourse.bass as bass

import ml_dtypes

import concourse.tile as tile
from concourse import bacc, mybir
from concourse.bass_utils import run_bass_kernel_spmd

# Problem constants (hardcoded per harness contract).
B, S_TOK, IN_F, OUT_F, GROUP = 8, 512, 4096, 4096, 64
T = B * S_TOK                # 4096 tokens
NCORES = 8
TSPLIT, OSPLIT = 2, 4        # core c = 4*h + q
TLOC = T // TSPLIT           # 2048 tokens per core
OC = OUT_F // OSPLIT         # 1024 output features per core
NG = IN_F * OUT_F // GROUP   # 262144 quant groups

NF8 = 1024                   # contraction columns computed in fp8 (multiple of 256)
NP8 = NF8 // 128             # fp8 i-planes (even)
NP16 = (IN_F - NF8) // 128   # fp16 i-planes
NPL = IN_F // 128            # 32 total i-planes

XSCALE = 16.0                # x pre-scale (power of 2, lossless in fp16)
WSCALE = 1024.0              # W pre-scale
DRAIN = 1.0 / (XSCALE * WSCALE)

TCHUNK = 256                 # tokens per psum round -> 4 banks of [128, 512]
NCH = TLOC // TCHUNK         # 8 chunks

F16 = mybir.dt.float16
F32 = mybir.dt.float32
F8 = mybir.dt.float8e4
E4M3 = ml_dtypes.float8_e4m3


def _trace_body(nc):
    Alu = mybir.AluOpType
    DR = mybir.MatmulPerfMode.DoubleRow
    # x16 ships pre-blocked so every DMA is fully contiguous per partition
    # (24KB lines); strided token-slicing would yield 0.5-1KB descriptor
    # lines that crawl through the DMA queues.
    x16a_d = nc.dram_tensor("x16a", [128, NP16, 512], F16, kind="ExternalInput")
    x16b = nc.dram_tensor("x16b", [TLOC // TCHUNK - 2, 128, NP16, TCHUNK],
                          F16, kind="ExternalInput")
    x8 = nc.dram_tensor("x8", [128, NP8, TLOC], F8, kind="ExternalInput")
    wqt = nc.dram_tensor("wqt", [128, NPL, OC], F8, kind="ExternalInput")
    zzt = nc.dram_tensor("zzt", [128, NPL, 64], F16, kind="ExternalInput")
    sst = nc.dram_tensor("sst", [128, NPL, 64], F16, kind="ExternalInput")
    bias_b = nc.dram_tensor("bias_b", [128, OC], F32, kind="ExternalInput")
    out = nc.dram_tensor("out", [TLOC, OC], F32, kind="ExternalOutput")

    TA = 512                  # phase-A token span (tokens 0:TA, 8 psum banks)
    NOB = OC // 512

    with tile.TileContext(nc) as tc:
        with (
            tc.tile_pool(name="res", bufs=1) as res,
            tc.tile_pool(name="wqp", bufs=6) as wqp,
            tc.tile_pool(name="tmpp", bufs=4) as tmpp,
            tc.tile_pool(name="xcp", bufs=2) as xcp,
            tc.tile_pool(name="outp", bufs=4) as outp,
            tc.tile_pool(name="psp", bufs=8, space=bass.MemorySpace.PSUM) as psp,
        ):
            # --- resident tensors ---
            # Queue roles: sync = pure input pump (zz, wq, x16a, all x16
            # chunks -- nothing on it ever waits except pool pacing);
            # scalar = ss + casts + ALL output stores; gpsimd (slow SWDGE)
            # = x8/bias only, needed late.
            zz_sb = res.tile([128, NPL, 64], F16)
            ss_sb = res.tile([128, NPL, 64], F16)
            nc.scalar.dma_start(zz_sb[:, 8:16, :], zzt[:, 8:16, :])
            nc.scalar.dma_start(ss_sb[:, 8:16, :], sst[:, 8:16, :])
            x8_sb = res.tile([128, NP8, TLOC], F8)
            bias_sb = res.tile([128, OC], F32)
            w8t = res.tile([128, NP8, OC], F8)
            w16t = res.tile([128, NP16, OC], F16)
            x16a = res.tile([128, NP16, TA], F16)
            dum = res.tile([128, 512], F16)
            nc.vector.memset(dum[:], 0.0)
            # wq stays fp8 in HBM (half the critical-path DMA) and lands in
            # one resident tile via big sub-DMAs -- no pool recycling, so no
            # DMA trigger ever waits.  Per-plane fp8->f16 casts on the
            # scalar engine (1.15us) feed the 2x-mode vector TTs (0.69us).
            # Interleave wq sub-DMAs with x16a quarters in consumption order.
            wq_all = res.tile([128, NPL, OC], F8)
            nc.sync.dma_start(wq_all[:, 8:10, :], wqt[:, 8:10, :])
            nc.sync.dma_start(x16a[:, 0:2, :], x16a_d[:, 0:2, :])
            nc.sync.dma_start(wq_all[:, 10:12, :], wqt[:, 10:12, :])
            nc.sync.dma_start(wq_all[:, 12:16, :], wqt[:, 12:16, :])
            nc.sync.dma_start(x16a[:, 2:6, :], x16a_d[:, 2:6, :])
            nc.sync.dma_start(wq_all[:, 16:20, :], wqt[:, 16:20, :])
            nc.sync.dma_start(x16a[:, 6:12, :], x16a_d[:, 6:12, :])
            nc.sync.dma_start(wq_all[:, 20:24, :], wqt[:, 20:24, :])
            nc.sync.dma_start(x16a[:, 12:NP16, :], x16a_d[:, 12:NP16, :])
            for j0 in (24, 28, 0, 4):
                nc.sync.dma_start(wq_all[:, j0:j0 + 4, :], wqt[:, j0:j0 + 4, :])
            for lo, hi in ((16, 32), (0, 8)):
                nc.scalar.dma_start(zz_sb[:, lo:hi, :], zzt[:, lo:hi, :])
                nc.scalar.dma_start(ss_sb[:, lo:hi, :], sst[:, lo:hi, :])

            def dequant(j):
                """(Q - z) * s for i-plane j: scalar cast + vector TTs."""
                wq16 = wqp.tile([128, OC], F16, tag="wq16", bufs=4,
                                name=f"wq16_{j}")
                nc.scalar.copy(wq16[:], wq_all[:, j, :])
                zb = zz_sb[:, j, :].unsqueeze(1).broadcast_to([128, OC // 64, 64])
                sb_ = ss_sb[:, j, :].unsqueeze(1).broadcast_to([128, OC // 64, 64])
                wq3 = wq16[:, :].rearrange("p (r m) -> p r m", m=64)
                tmp = tmpp.tile([128, OC], F16, tag="tmp", name=f"tmp{j}")
                tmp3 = tmp[:, :].rearrange("p (r m) -> p r m", m=64)
                nc.vector.tensor_tensor(tmp3, wq3, zb, op=Alu.subtract)
                if j < NP8:
                    # direct fp8-out TT runs at 1x (1.2us) but beats any
                    # cast chain (gpsimd casts cost ~3.9us each).
                    o3 = w8t[:, j, :].rearrange("p (r m) -> p r m", m=64)
                else:
                    o3 = w16t[:, j - NP8, :].rearrange("p (r m) -> p r m", m=64)
                nc.vector.tensor_tensor(o3, tmp3, sb_, op=Alu.mult)

            def drain(ps, t_lo, ob):
                o_sb = outp.tile([128, 512], F32, tag="o")
                nc.vector.scalar_tensor_tensor(
                    o_sb[:], ps[:], DRAIN,
                    bias_sb[:, ob * 512:(ob + 1) * 512],
                    op0=Alu.mult, op1=Alu.add,
                )
                nc.scalar.dma_start(
                    out[t_lo:t_lo + 128, ob * 512:(ob + 1) * 512], o_sb[:])

            # --- phase A: tokens 0:TA, plane-major (PE follows the dequant
            # stream at 8 MMs per plane instead of starving at 2) ---
            psA = [[psp.tile([128, 512], F32, tag="ps", name=f"psA{tt}_{ob}")
                    for ob in range(NOB)] for tt in range(TA // 128)]
            # fp16 planes stream first (consumption-rate matched); fp8
            # planes dequant mid-stream so their MMs are ready well before
            # they close phase A.
            dequant(NP8 + 0)
            dequant(NP8 + 1)
            for jj in range(2, NP16):
                dequant(NP8 + jj)
                if jj == 6:
                    nc.gpsimd.dma_start(x8_sb[:], x8[:])
                    nc.gpsimd.dma_start(bias_sb[:], bias_b[:])
            for j in range(NP8):
                dequant(j)   # fp8 planes last = exact PE consumption order
            # PE warm-up: HAM needs ~3.4us of activity to unthrottle; run
            # dummy matmuls on a zero tile while the first W planes dequant.
            for _ in range(12):
                nc.tensor.matmul(
                    psA[0][0][:], dum[:, 0:128], dum[:, :],
                    start=True, stop=True, skip_group_check=True,
                )
            for it in range(NP16):
                for tt in range(TA // 128):
                    for ob in range(NOB):
                        nc.tensor.matmul(
                            psA[tt][ob][:],
                            x16a[:, it, tt * 128:tt * 128 + 128],
                            w16t[:, it, ob * 512:(ob + 1) * 512],
                            start=(it == 0), stop=False,
                        )
            for pp in range(0, NP8, 2):
                for tt in range(TA // 128):
                    for ob in range(NOB):
                        nc.tensor.matmul(
                            psA[tt][ob][:],
                            x8_sb[:, pp:pp + 2, tt * 128:tt * 128 + 128],
                            w8t[:, pp:pp + 2, ob * 512:(ob + 1) * 512],
                            start=False, stop=(pp == NP8 - 2),
                            perf_mode=DR,
                        )
            # pre-issue the first phase-B chunk DMAs so they aren't stuck
            # behind the phase-A drain-store triggers in sync's program
            xcs = {}
            for ch in range(TA // TCHUNK, min(TA // TCHUNK + 2, NCH)):
                xcs[ch] = xcp.tile([128, NP16, TCHUNK], F16, tag="xc",
                                   name=f"xc{ch}")
                nc.sync.dma_start(xcs[ch][:], x16b[ch - 2])
            for tt in range(TA // 128):
                for ob in range(NOB):
                    drain(psA[tt][ob], tt * 128, ob)

            # --- phase B: remaining tokens, token-major ---
            for ch in range(TA // TCHUNK, NCH):
                if ch in xcs:
                    xc = xcs[ch]
                else:
                    xc = xcp.tile([128, NP16, TCHUNK], F16, tag="xc",
                                  name=f"xc{ch}")
                    nc.sync.dma_start(xc[:], x16b[ch - 2])
                for tt in range(TCHUNK // 128):
                    t0 = ch * TCHUNK + tt * 128
                    psums = [
                        psp.tile([128, 512], F32, tag="ps", name=f"ps{ch}_{tt}_{ob}")
                        for ob in range(NOB)
                    ]
                    for pp in range(0, NP8, 2):
                        for ob in range(NOB):
                            nc.tensor.matmul(
                                psums[ob][:],
                                x8_sb[:, pp:pp + 2, t0:t0 + 128],
                                w8t[:, pp:pp + 2, ob * 512:(ob + 1) * 512],
                                start=(pp == 0), stop=False,
                                perf_mode=DR,
                            )
                    last_tile = (ch == NCH - 1 and tt == TCHUNK // 128 - 1)
                    if last_tile:
                        # ob-major: bank ob0 closes ~5us early so its
                        # drain+store overlaps ob1's matmuls (tail shave)
                        for ob in range(NOB):
                            for it in range(NP16):
                                nc.tensor.matmul(
                                    psums[ob][:],
                                    xc[:, it, tt * 128:tt * 128 + 128],
                                    w16t[:, it, ob * 512:(ob + 1) * 512],
                                    start=False, stop=(it == NP16 - 1),
                                )
                            drain(psums[ob], t0, ob)
                    else:
                        for it in range(NP16):
                            for ob in range(NOB):
                                nc.tensor.matmul(
                                    psums[ob][:],
                                    xc[:, it, tt * 128:tt * 128 + 128],
                                    w16t[:, it, ob * 512:(ob + 1) * 512],
                                    start=False, stop=(it == NP16 - 1),
                                )
                    if not last_tile:
                        for ob in range(NOB):
                            drain(psums[ob], t0, ob)


_CACHED_NC = None


def _get_nc():
    global _CACHED_NC
    if _CACHED_NC is None:
        nc = bacc.Bacc("TRN2", target_bir_lowering=False, debug=False)
        _trace_body(nc)
        nc.compile()
        _CACHED_NC = nc
    return _CACHED_NC


def _plane_pack(a):
    """[TLOC, n*128] -> [128, n, TLOC] with i = j*128 + p."""
    tl, nf = a.shape
    return np.ascontiguousarray(a.reshape(tl, nf // 128, 128).transpose(2, 1, 0))


def make_in_maps(x, W_q, scale, zero, bias):
    """Shard the full inputs into the 8 per-core input maps."""
    xs = np.asarray(x).reshape(T, IN_F).astype(np.float32) * XSCALE
    W_q = np.asarray(W_q)
    # zero/scale in [i, m=oc%64] layout, plane-packed to [128, NPL, 64].
    zz = np.asarray(zero).reshape(GROUP, IN_F).T.astype(np.float16)
    ss = (np.asarray(scale).reshape(GROUP, IN_F).T * WSCALE).astype(np.float16)
    zz_t = np.ascontiguousarray(zz.reshape(NPL, 128, 64).transpose(1, 0, 2))
    ss_t = np.ascontiguousarray(ss.reshape(NPL, 128, 64).transpose(1, 0, 2))
    bias = np.asarray(bias).astype(np.float32)

    x16a_h, x16b_h, x8_h = [], [], []
    for h in range(TSPLIT):
        xh = xs[h * TLOC:(h + 1) * TLOC]
        x16 = _plane_pack(xh[:, NF8:]).astype(np.float16)  # [128, NP16, TLOC]
        x16a_h.append(np.ascontiguousarray(x16[:, :, 0:512]))
        x16b_h.append(np.ascontiguousarray(
            x16.reshape(128, NP16, NCH, TCHUNK)[:, :, 2:, :].transpose(2, 0, 1, 3)))
        x8_h.append(_plane_pack(xh[:, :NF8]).astype(E4M3))

    wqt_q, bias_q = [], []
    for q in range(OSPLIT):
        g0 = q * (OC // 64)          # first unpacked row for this quarter
        if g0 < GROUP // 2:
            rows = ((W_q[g0:g0 + OC // 64] >> 4) & 15)
        else:
            rows = (W_q[g0 - GROUP // 2:g0 - GROUP // 2 + OC // 64] & 15)
        # rows: [16, NG] -> Q[oc_l, i] with oc_l = g_l*64 + m, col n = m*4096 + i
        Qm = rows.reshape(OC // 64, 64, IN_F).reshape(OC, IN_F)
        QT = Qm.T.astype(np.float32)                    # [i, oc_l]
        wqt_q.append(np.ascontiguousarray(
            QT.reshape(NPL, 128, OC).transpose(1, 0, 2)).astype(E4M3))
        bias_q.append(np.ascontiguousarray(
            np.broadcast_to(bias[OC * q:OC * (q + 1)], (128, OC))))

    in_maps = []
    for c in range(NCORES):
        h, q = c // OSPLIT, c % OSPLIT
        in_maps.append({
            "x16a": x16a_h[h],
            "x16b": x16b_h[h],
            "x8": x8_h[h],
            "wqt": wqt_q[q],
            "zzt": zz_t,
            "sst": ss_t,
            "bias_b": bias_q[q],
        })
    return in_maps


def assemble(results):
    """results: list of per-core {"out": [TLOC, OC] f32} -> [B, S, OUT_F] f32."""
    full = np.empty((T, OUT_F), np.float32)
    for c in range(NCORES):
        h, q = c // OSPLIT, c % OSPLIT
        full[h * TLOC:(h + 1) * TLOC, q * OC:(q + 1) * OC] = results[c]["out"]
    return full.reshape(B, S_TOK, OUT_F)


def kernel(x, W_q, scale, zero, bias):
    nc = _get_nc()
    in_maps = make_in_maps(x, W_q, scale, zero, bias)
    res = run_bass_kernel_spmd(nc, in_maps, core_ids=list(range(NCORES)))
    return assemble(res.results)


if __name__ == "__main__":
    # Quick CoreSim check of cores 0 and 7 against a numpy reference.
    from concourse.bass_interp import CoreSim

    rng = np.random.default_rng(0)
    x = rng.standard_normal((B, S_TOK, IN_F), dtype=np.float32)
    W_q = rng.integers(0, 256, (GROUP // 2, NG)).astype(np.int32)
    scale = rng.uniform(1e-3, 1e-2, (1, NG)).astype(np.float32)
    zero = rng.uniform(0.0, 15.0, (1, NG)).astype(np.float32)
    bias = (rng.standard_normal(OUT_F) * 0.01).astype(np.float32)

    hi = (W_q >> 4) & 0xF
    lo = W_q & 0xF
    W_p = np.concatenate([hi, lo], axis=0).astype(np.float32)
    W_est = ((W_p - zero) * scale).reshape(OUT_F, IN_F)
    ref = x.reshape(T, IN_F) @ W_est.T + bias

    nc = _get_nc()
    in_maps = make_in_maps(x, W_q, scale, zero, bias)
    for core in (0, 7):
        sim = CoreSim(nc, trace=False)
        for k, v in in_maps[core].items():
            sim.tensor(k)[:] = v
        sim.simulate(check_with_hw=False)
        got = np.asarray(sim.tensor("out"))
        h, q = core // OSPLIT, core % OSPLIT
        exp = ref[h * TLOC:(h + 1) * TLOC, q * OC:(q + 1) * OC]
        err = np.abs(got - exp)
        rel = err.max() / np.abs(ref).max()
        print(f"core {core}: max abs err {err.max():.3e}  "
              f"rel (vs global absmax) {rel:.3e}  mean abs {err.mean():.3e}")


# revision 41
# speedup vs baseline: 1.0763x; 1.0397x over previous
"""HQQ 4-bit quantized linear on 8 trn2 NeuronCores (hybrid fp8/fp16).

Computation: out[b,s,o] = sum_i x[b,s,i] * W_est[o,i] + bias[o], where
W_est = ((unpack4bit(W_q) - zero) * scale).reshape(4096, 4096).

Sharding (2 token-halves x 4 output-quarters): core c = 4*h + q computes
out[2048h : 2048h+2048, 1024q : 1024q+1024].  This halves the replicated-x
DMA per core vs pure column-parallel (the PE stream is identical either
way; the baseline's mid-kernel stalls were x-DMA starvation).

Precision: the contraction dim i is split NF8 columns fp8-e4m3 (DoubleRow,
2 MACs/cycle) + the rest fp16.  Everything is scaled by 2^14 (x by 16, W
by 1024 -- lossless powers of 2 for the fp16 side) so fp8 and fp16 matmuls
accumulate into the SAME fp32 PSUM bank; one fused DVE op rescales and
adds bias on drain.  fp8 W values sit in e4m3's normal range (|W|*1024 up
to ~157 < 240); measured end-to-end rel err ~1.5e-2 < 2e-2 gate.

Dequant happens directly in transposed [i, oc] layout (no PE transposes,
no PSUM round-trip): host ships the 4-bit codes Q as e4m3 (integers 0..15
are exact in e4m3) already transposed, plus zero/scale in [i, oc%64]
layout; the device does (Q - z) * s with stride-0 broadcast APs along the
64-periodic oc axis, f16 arithmetic (2x DVE rate), split across the
vector and gpsimd engines.

Device program per core:
  1. Dequant 32 i-planes: NF8/128 planes -> W8T e4m3 [128, *, 1024],
     rest -> W16T f16.
  2. Main: 8 chunks of 256 tokens; per 128-token tile: 2x(NF8/256) fp8
     DoubleRow MMs (stationary x8 plane-pair, moving W8T [128,2,512]) +
     2x24 fp16 MMs (stationary x16 [128,128], moving W16T [128,512]),
     all accumulating into psum[t 128, oc 512]; drain = fused
     (psum * 2^-14) + bias on DVE, stores [128, 512] f32.
"""

import sys

import numpy as np

try:
    import concourse.bass as bass
except ImportError:  # fresh grading dir: fall back to the repo checkout
    for _p in ("/opt/trn_rl_repo", "/root/.axon_site/_ro/trn_rl_repo"):
        if _p not in sys.path:
            sys.path.insert(0, _p)
    import concourse.bass as bass

import ml_dtypes

import concourse.tile as tile
from concourse import bacc, mybir
from concourse.bass_utils import run_bass_kernel_spmd

# Problem constants (hardcoded per harness contract).
B, S_TOK, IN_F, OUT_F, GROUP = 8, 512, 4096, 4096, 64
T = B * S_TOK                # 4096 tokens
NCORES = 8
TSPLIT, OSPLIT = 2, 4        # core c = 4*h + q
TLOC = T // TSPLIT           # 2048 tokens per core
OC = OUT_F // OSPLIT         # 1024 output features per core
NG = IN_F * OUT_F // GROUP   # 262144 quant groups

NF8 = 1536                   # contraction columns computed in fp8 (multiple of 256)
NP8 = NF8 // 128             # fp8 i-planes (even)
NP16 = (IN_F - NF8) // 128   # fp16 i-planes
NPL = IN_F // 128            # 32 total i-planes

XSCALE = 16.0                # x pre-scale (power of 2, lossless in fp16)
WSCALE = 1024.0              # W pre-scale
DRAIN = 1.0 / (XSCALE * WSCALE)

TCHUNK = 256                 # tokens per psum round -> 4 banks of [128, 512]
NCH = TLOC // TCHUNK         # 8 chunks

F16 = mybir.dt.float16
F32 = mybir.dt.float32
F8 = mybir.dt.float8e4
E4M3 = ml_dtypes.float8_e4m3


def _trace_body(nc):
    Alu = mybir.AluOpType
    DR = mybir.MatmulPerfMode.DoubleRow
    # x16 ships pre-blocked so every DMA is fully contiguous per partition
    # (24KB lines); strided token-slicing would yield 0.5-1KB descriptor
    # lines that crawl through the DMA queues.
    x16a_d = nc.dram_tensor("x16a", [128, NP16, 512], F16, kind="ExternalInput")
    x16b = nc.dram_tensor("x16b", [TLOC // TCHUNK - 2, 128, NP16, TCHUNK],
                          F16, kind="ExternalInput")
    x8 = nc.dram_tensor("x8", [128, NP8, TLOC], F8, kind="ExternalInput")
    wqt = nc.dram_tensor("wqt", [128, NPL, OC], F8, kind="ExternalInput")
    zzt = nc.dram_tensor("zzt", [128, NPL, 64], F16, kind="ExternalInput")
    sst = nc.dram_tensor("sst", [128, NPL, 64], F16, kind="ExternalInput")
    bias_b = nc.dram_tensor("bias_b", [128, OC], F32, kind="ExternalInput")
    out = nc.dram_tensor("out", [TLOC, OC], F32, kind="ExternalOutput")

    TA = 512                  # phase-A token span (tokens 0:TA, 8 psum banks)
    NOB = OC // 512

    with tile.TileContext(nc) as tc:
        with (
            tc.tile_pool(name="res", bufs=1) as res,
            tc.tile_pool(name="wqp", bufs=6) as wqp,
            tc.tile_pool(name="tmpp", bufs=4) as tmpp,
            tc.tile_pool(name="xcp", bufs=2) as xcp,
            tc.tile_pool(name="outp", bufs=4) as outp,
            tc.tile_pool(name="psp", bufs=8, space=bass.MemorySpace.PSUM) as psp,
        ):
            # --- resident tensors ---
            # Queue roles: sync = pure input pump (zz, wq, x16a, all x16
            # chunks -- nothing on it ever waits except pool pacing);
            # scalar = ss + casts + ALL output stores; gpsimd (slow SWDGE)
            # = x8/bias only, needed late.
            zz_sb = res.tile([128, NPL, 64], F16)
            ss_sb = res.tile([128, NPL, 64], F16)
            nc.scalar.dma_start(zz_sb[:, 8:16, :], zzt[:, 8:16, :])
            nc.scalar.dma_start(ss_sb[:, 8:16, :], sst[:, 8:16, :])
            x8_sb = res.tile([128, NP8, TLOC], F8)
            bias_sb = res.tile([128, OC], F32)
            w8t = res.tile([128, NP8, OC], F8)
            w16t = res.tile([128, NP16, OC], F16)
            x16a = res.tile([128, NP16, TA], F16)
            dum = res.tile([128, 512], F16)
            nc.vector.memset(dum[:], 0.0)
            # wq stays fp8 in HBM (half the critical-path DMA) and lands in
            # one resident tile via big sub-DMAs -- no pool recycling, so no
            # DMA trigger ever waits.  Per-plane fp8->f16 casts on the
            # scalar engine (1.15us) feed the 2x-mode vector TTs (0.69us).
            # Interleave wq sub-DMAs with x16a quarters in consumption order.
            wq_all = res.tile([128, NPL, OC], F8)
            nc.sync.dma_start(wq_all[:, 8:10, :], wqt[:, 8:10, :])
            nc.sync.dma_start(x16a[:, 0:2, :], x16a_d[:, 0:2, :])
            nc.sync.dma_start(wq_all[:, 10:12, :], wqt[:, 10:12, :])
            nc.sync.dma_start(wq_all[:, 12:16, :], wqt[:, 12:16, :])
            nc.sync.dma_start(x16a[:, 2:6, :], x16a_d[:, 2:6, :])
            nc.sync.dma_start(wq_all[:, 16:20, :], wqt[:, 16:20, :])
            nc.sync.dma_start(x16a[:, 6:12, :], x16a_d[:, 6:12, :])
            nc.sync.dma_start(wq_all[:, 20:24, :], wqt[:, 20:24, :])
            nc.sync.dma_start(x16a[:, 12:NP16, :], x16a_d[:, 12:NP16, :])
            for j0 in (24, 28, 0, 4):
                nc.sync.dma_start(wq_all[:, j0:j0 + 4, :], wqt[:, j0:j0 + 4, :])
            for lo, hi in ((16, 32), (0, 8)):
                nc.scalar.dma_start(zz_sb[:, lo:hi, :], zzt[:, lo:hi, :])
                nc.scalar.dma_start(ss_sb[:, lo:hi, :], sst[:, lo:hi, :])

            def dequant(j):
                """(Q - z) * s for i-plane j: scalar cast + vector TTs."""
                wq16 = wqp.tile([128, OC], F16, tag="wq16", bufs=4,
                                name=f"wq16_{j}")
                nc.scalar.copy(wq16[:], wq_all[:, j, :])
                zb = zz_sb[:, j, :].unsqueeze(1).broadcast_to([128, OC // 64, 64])
                sb_ = ss_sb[:, j, :].unsqueeze(1).broadcast_to([128, OC // 64, 64])
                wq3 = wq16[:, :].rearrange("p (r m) -> p r m", m=64)
                tmp = tmpp.tile([128, OC], F16, tag="tmp", name=f"tmp{j}")
                tmp3 = tmp[:, :].rearrange("p (r m) -> p r m", m=64)
                nc.vector.tensor_tensor(tmp3, wq3, zb, op=Alu.subtract)
                if j < NP8:
                    # direct fp8-out TT runs at 1x on vector (1.2us);
                    # alternate onto the idle gpsimd (2.3us but concurrent;
                    # fp8 planes are consumed last, so slack is ample)
                    o3 = w8t[:, j, :].rearrange("p (r m) -> p r m", m=64)
                    eng = nc.gpsimd if j % 2 == 0 else nc.vector
                else:
                    o3 = w16t[:, j - NP8, :].rearrange("p (r m) -> p r m", m=64)
                    eng = nc.vector
                eng.tensor_tensor(o3, tmp3, sb_, op=Alu.mult)

            def drain(ps, t_lo, ob):
                o_sb = outp.tile([128, 512], F32, tag="o")
                nc.vector.scalar_tensor_tensor(
                    o_sb[:], ps[:], DRAIN,
                    bias_sb[:, ob * 512:(ob + 1) * 512],
                    op0=Alu.mult, op1=Alu.add,
                )
                nc.scalar.dma_start(
                    out[t_lo:t_lo + 128, ob * 512:(ob + 1) * 512], o_sb[:])

            # --- phase A: tokens 0:TA, plane-major (PE follows the dequant
            # stream at 8 MMs per plane instead of starving at 2) ---
            psA = [[psp.tile([128, 512], F32, tag="ps", name=f"psA{tt}_{ob}")
                    for ob in range(NOB)] for tt in range(TA // 128)]
            # fp16 planes stream first (consumption-rate matched); fp8
            # planes dequant mid-stream so their MMs are ready well before
            # they close phase A.
            dequant(NP8 + 0)
            dequant(NP8 + 1)
            for jj in range(2, NP16):
                dequant(NP8 + jj)
                if jj == 6:
                    for pp in range(0, NP8, 2):
                        nc.sync.dma_start(x8_sb[:, pp:pp + 2, :],
                                          x8[:, pp:pp + 2, :])
                    nc.gpsimd.dma_start(bias_sb[:], bias_b[:])
            for j in range(NP8):
                dequant(j)   # fp8 planes last = exact PE consumption order
            # PE warm-up: HAM needs ~3.4us of activity to unthrottle; run
            # dummy matmuls on a zero tile while the first W planes dequant.
            for _ in range(12):
                nc.tensor.matmul(
                    psA[0][0][:], dum[:, 0:128], dum[:, :],
                    start=True, stop=True, skip_group_check=True,
                )
            for it in range(NP16):
                for tt in range(TA // 128):
                    for ob in range(NOB):
                        nc.tensor.matmul(
                            psA[tt][ob][:],
                            x16a[:, it, tt * 128:tt * 128 + 128],
                            w16t[:, it, ob * 512:(ob + 1) * 512],
                            start=(it == 0), stop=False,
                        )
            for pp in range(0, NP8, 2):
                for tt in range(TA // 128):
                    for ob in range(NOB):
                        nc.tensor.matmul(
                            psA[tt][ob][:],
                            x8_sb[:, pp:pp + 2, tt * 128:tt * 128 + 128],
                            w8t[:, pp:pp + 2, ob * 512:(ob + 1) * 512],
                            start=False, stop=(pp == NP8 - 2),
                            perf_mode=DR,
                        )
            # pre-issue the first phase-B chunk DMAs so they aren't stuck
            # behind the phase-A drain-store triggers in sync's program
            xcs = {}
            for ch in range(TA // TCHUNK, min(TA // TCHUNK + 2, NCH)):
                xcs[ch] = xcp.tile([128, NP16, TCHUNK], F16, tag="xc",
                                   name=f"xc{ch}")
                nc.sync.dma_start(xcs[ch][:], x16b[ch - 2])
            for tt in range(TA // 128):
                for ob in range(NOB):
                    drain(psA[tt][ob], tt * 128, ob)

            # --- phase B: remaining tokens, token-major ---
            for ch in range(TA // TCHUNK, NCH):
                if ch in xcs:
                    xc = xcs[ch]
                else:
                    xc = xcp.tile([128, NP16, TCHUNK], F16, tag="xc",
                                  name=f"xc{ch}")
                    nc.sync.dma_start(xc[:], x16b[ch - 2])
                for tt in range(TCHUNK // 128):
                    t0 = ch * TCHUNK + tt * 128
                    psums = [
                        psp.tile([128, 512], F32, tag="ps", name=f"ps{ch}_{tt}_{ob}")
                        for ob in range(NOB)
                    ]
                    for pp in range(0, NP8, 2):
                        for ob in range(NOB):
                            nc.tensor.matmul(
                                psums[ob][:],
                                x8_sb[:, pp:pp + 2, t0:t0 + 128],
                                w8t[:, pp:pp + 2, ob * 512:(ob + 1) * 512],
                                start=(pp == 0), stop=False,
                                perf_mode=DR,
                            )
                    last_tile = (ch == NCH - 1 and tt == TCHUNK // 128 - 1)
                    if last_tile:
                        # ob-major: bank ob0 closes ~5us early so its
                        # drain+store overlaps ob1's matmuls (tail shave)
                        for ob in range(NOB):
                            for it in range(NP16):
                                nc.tensor.matmul(
                                    psums[ob][:],
                                    xc[:, it, tt * 128:tt * 128 + 128],
                                    w16t[:, it, ob * 512:(ob + 1) * 512],
                                    start=False, stop=(it == NP16 - 1),
                                )
                            drain(psums[ob], t0, ob)
                    else:
                        for it in range(NP16):
                            for ob in range(NOB):
                                nc.tensor.matmul(
                                    psums[ob][:],
                                    xc[:, it, tt * 128:tt * 128 + 128],
                                    w16t[:, it, ob * 512:(ob + 1) * 512],
                                    start=False, stop=(it == NP16 - 1),
                                )
                    if not last_tile:
                        for ob in range(NOB):
                            drain(psums[ob], t0, ob)


_CACHED_NC = None


def _get_nc():
    global _CACHED_NC
    if _CACHED_NC is None:
        nc = bacc.Bacc("TRN2", target_bir_lowering=False, debug=False)
        _trace_body(nc)
        nc.compile()
        _CACHED_NC = nc
    return _CACHED_NC


def _plane_pack(a):
    """[TLOC, n*128] -> [128, n, TLOC] with i = j*128 + p."""
    tl, nf = a.shape
    return np.ascontiguousarray(a.reshape(tl, nf // 128, 128).transpose(2, 1, 0))


def make_in_maps(x, W_q, scale, zero, bias):
    """Shard the full inputs into the 8 per-core input maps."""
    xs = np.asarray(x).reshape(T, IN_F).astype(np.float32) * XSCALE
    W_q = np.asarray(W_q)
    # zero/scale in [i, m=oc%64] layout, plane-packed to [128, NPL, 64].
    zz = np.asarray(zero).reshape(GROUP, IN_F).T.astype(np.float16)
    ss = (np.asarray(scale).reshape(GROUP, IN_F).T * WSCALE).astype(np.float16)
    zz_t = np.ascontiguousarray(zz.reshape(NPL, 128, 64).transpose(1, 0, 2))
    ss_t = np.ascontiguousarray(ss.reshape(NPL, 128, 64).transpose(1, 0, 2))
    bias = np.asarray(bias).astype(np.float32)

    x16a_h, x16b_h, x8_h = [], [], []
    for h in range(TSPLIT):
        xh = xs[h * TLOC:(h + 1) * TLOC]
        x16 = _plane_pack(xh[:, NF8:]).astype(np.float16)  # [128, NP16, TLOC]
        x16a_h.append(np.ascontiguousarray(x16[:, :, 0:512]))
        x16b_h.append(np.ascontiguousarray(
            x16.reshape(128, NP16, NCH, TCHUNK)[:, :, 2:, :].transpose(2, 0, 1, 3)))
        x8_h.append(_plane_pack(xh[:, :NF8]).astype(E4M3))

    wqt_q, bias_q = [], []
    for q in range(OSPLIT):
        g0 = q * (OC // 64)          # first unpacked row for this quarter
        if g0 < GROUP // 2:
            rows = ((W_q[g0:g0 + OC // 64] >> 4) & 15)
        else:
            rows = (W_q[g0 - GROUP // 2:g0 - GROUP // 2 + OC // 64] & 15)
        # rows: [16, NG] -> Q[oc_l, i] with oc_l = g_l*64 + m, col n = m*4096 + i
        Qm = rows.reshape(OC // 64, 64, IN_F).reshape(OC, IN_F)
        QT = Qm.T.astype(np.float32)                    # [i, oc_l]
        wqt_q.append(np.ascontiguousarray(
            QT.reshape(NPL, 128, OC).transpose(1, 0, 2)).astype(E4M3))
        bias_q.append(np.ascontiguousarray(
            np.broadcast_to(bias[OC * q:OC * (q + 1)], (128, OC))))

    in_maps = []
    for c in range(NCORES):
        h, q = c // OSPLIT, c % OSPLIT
        in_maps.append({
            "x16a": x16a_h[h],
            "x16b": x16b_h[h],
            "x8": x8_h[h],
            "wqt": wqt_q[q],
            "zzt": zz_t,
            "sst": ss_t,
            "bias_b": bias_q[q],
        })
    return in_maps


def assemble(results):
    """results: list of per-core {"out": [TLOC, OC] f32} -> [B, S, OUT_F] f32."""
    full = np.empty((T, OUT_F), np.float32)
    for c in range(NCORES):
        h, q = c // OSPLIT, c % OSPLIT
        full[h * TLOC:(h + 1) * TLOC, q * OC:(q + 1) * OC] = results[c]["out"]
    return full.reshape(B, S_TOK, OUT_F)


def kernel(x, W_q, scale, zero, bias):
    nc = _get_nc()
    in_maps = make_in_maps(x, W_q, scale, zero, bias)
    res = run_bass_kernel_spmd(nc, in_maps, core_ids=list(range(NCORES)))
    return assemble(res.results)


if __name__ == "__main__":
    # Quick CoreSim check of cores 0 and 7 against a numpy reference.
    from concourse.bass_interp import CoreSim

    rng = np.random.default_rng(0)
    x = rng.standard_normal((B, S_TOK, IN_F), dtype=np.float32)
    W_q = rng.integers(0, 256, (GROUP // 2, NG)).astype(np.int32)
    scale = rng.uniform(1e-3, 1e-2, (1, NG)).astype(np.float32)
    zero = rng.uniform(0.0, 15.0, (1, NG)).astype(np.float32)
    bias = (rng.standard_normal(OUT_F) * 0.01).astype(np.float32)

    hi = (W_q >> 4) & 0xF
    lo = W_q & 0xF
    W_p = np.concatenate([hi, lo], axis=0).astype(np.float32)
    W_est = ((W_p - zero) * scale).reshape(OUT_F, IN_F)
    ref = x.reshape(T, IN_F) @ W_est.T + bias

    nc = _get_nc()
    in_maps = make_in_maps(x, W_q, scale, zero, bias)
    for core in (0, 7):
        sim = CoreSim(nc, trace=False)
        for k, v in in_maps[core].items():
            sim.tensor(k)[:] = v
        sim.simulate(check_with_hw=False)
        got = np.asarray(sim.tensor("out"))
        h, q = core // OSPLIT, core % OSPLIT
        exp = ref[h * TLOC:(h + 1) * TLOC, q * OC:(q + 1) * OC]
        err = np.abs(got - exp)
        rel = err.max() / np.abs(ref).max()
        print(f"core {core}: max abs err {err.max():.3e}  "
              f"rel (vs global absmax) {rel:.3e}  mean abs {err.mean():.3e}")
